# revision 2
# baseline (speedup 1.0000x reference)
"""Trainium2 Bass kernel for nn_LSM_IniReconNet.

The reference computes, per contiguous 16-element block of the signal,
z = W1 @ block then y = W2 @ z — i.e. a fixed 16x16 linear map
M = W2 @ W1 applied blockwise. This is pure streaming (memory-bound):
every element is read once, transformed by M, written once.

v2 strategy:
  * bf16 on the wire both directions (rel-err gate is 2e-2; bf16
    end-to-end lands ~4e-3), halving HBM traffic per core to
    4 MB in + 4 MB out.
  * The host lays each core's slice out as [128 partitions = signal
    position within a 128-superblock, free = (superblock, row)] so the
    contraction dim is already on partitions: the device needs NO
    transposes — just DMA in, one bf16 matmul per [128,512] chunk
    against the constant K = kron(I8, M.T), a PSUM->SBUF copy (casting
    back to bf16), and DMA out. The host inverts the permutation.
  * HWDGE DMAs: loads on nc.sync (SP ring), stores on nc.scalar (ACT
    ring) — separate rings, concurrent in/out streaming, ~0.6us fixed
    cost vs ~2us for the SWDGE path.

Sharding: pure data parallel — batch rows split across 8 cores, K
replicated.
"""

import sys

for _p in ("/opt/trn_rl_repo", "/root/.axon_site/_ro/trn_rl_repo"):
    if _p not in sys.path:
        sys.path.insert(0, _p)

import ml_dtypes
import numpy as np

import concourse.bass as bass
import concourse.mybir as mybir
from concourse.bass_utils import run_bass_kernel_spmd
from concourse.tile import TileContext

F32 = mybir.dt.float32
BF16 = mybir.dt.bfloat16
NPBF16 = np.dtype(ml_dtypes.bfloat16)

NB = 4096  # batch
H = 4096  # signal length
BLOCK = 16
SP = 8
N_CORES = 8
ROWS_PER_CORE = NB // N_CORES  # 512
NSUPER = H // 128  # 32 superblocks of 128 positions per row
NGROUPS = 4  # DMA granularity: 1 MB bf16 per group
CHUNKS_PER_GROUP = (NSUPER * ROWS_PER_CORE // 512) // NGROUPS  # 8
FREE = NSUPER * ROWS_PER_CORE  # 16384 free columns on chip

_NC_CACHE = {}


def _split_multi_waits(nc):
    """walrus codegen accepts at most one semaphore wait per instruction
    (beyond what same-queue elision removes). Tile attaches several — most
    notably on the kernel-tail drain. Hoist all but one wait onto wait-only
    NOPs placed immediately before the instruction on the same engine queue.
    """
    ctr = 0
    for fn in nc.m.functions:
        for blk in fn.blocks:
            old = list(blk.instructions)
            if not any(
                i.sync_info is not None and len(i.sync_info.on_wait) > 1 for i in old
            ):
                continue
            new = []
            for inst in old:
                si = inst.sync_info
                if si is not None and len(si.on_wait) > 1:
                    waits = list(si.on_wait)
                    for w in waits[:-1]:
                        ctr += 1
                        new.append(
                            mybir.InstNoOp(
                                name=f"I-waitsplit-{ctr}",
                                sync_info=mybir.SyncInfo(on_wait=[w], on_update=[]),
                                bass_nofuse=True,
                                engine=inst.engine,
                            )
                        )
                    inst.sync_info = mybir.SyncInfo(
                        on_wait=[waits[-1]], on_update=list(si.on_update)
                    )
                new.append(inst)
            blk.instructions = new
    return nc


def _build():
    """Per-core SPMD program.

    x: (128, FREE) bf16 — partition p holds position (128*c + p) of the
    signal for superblock c, free col c*512+n is batch row n.
    k: (128, 128) bf16 = kron(I8, M.T).  y: same layout as x.
    """
    nc = bass.Bass()
    x = nc.declare_dram_parameter("x", [128, FREE], BF16, isOutput=False)
    k = nc.declare_dram_parameter("k", [128, 128], BF16, isOutput=False)
    y = nc.declare_dram_parameter("y", [128, FREE], BF16, isOutput=True)
    gcols = FREE // NGROUPS  # 4096 free cols per DMA group

    with TileContext(nc) as tc:
        with (
            tc.tile_pool(name="kpool", bufs=1) as kp,
            tc.tile_pool(name="xin", bufs=3) as xin,
            tc.tile_pool(name="yout", bufs=4) as yp,
            tc.tile_pool(name="ps", bufs=8, space="PSUM") as pp,
        ):
            k_sb = kp.tile([128, 128], BF16)
            nc.sync.dma_start(out=k_sb[:], in_=k[:])
            # Warm-up matmul: consumes the K-DMA wait early.
            ps = pp.tile([128, 512], F32, tag="ps")
            nc.tensor.matmul(ps[:, :128], k_sb[:], k_sb[:], start=True, stop=True)
            for g in range(NGROUPS):
                xt = xin.tile([128, gcols], BF16)
                nc.sync.dma_start(out=xt[:], in_=x[:, g * gcols : (g + 1) * gcols])
                yt = yp.tile([128, gcols], BF16)
                for c in range(CHUNKS_PER_GROUP):
                    ps = pp.tile([128, 512], F32, tag="ps")
                    nc.tensor.matmul(
                        ps[:],
                        k_sb[:],
                        xt[:, c * 512 : (c + 1) * 512],
                        start=True,
                        stop=True,
                    )
                    nc.vector.tensor_copy(yt[:, c * 512 : (c + 1) * 512], ps[:])
                nc.scalar.dma_start(out=y[:, g * gcols : (g + 1) * gcols], in_=yt[:])
    return _split_multi_waits(nc)


def _get_nc():
    if "nc" not in _NC_CACHE:
        _NC_CACHE["nc"] = _build()
    return _NC_CACHE["nc"]


def _shard(x2d_bf16, i):
    """Core i's slice, permuted to device layout B[p, c*512+n] =
    x[n, 128c+p]."""
    xs = x2d_bf16[i * ROWS_PER_CORE : (i + 1) * ROWS_PER_CORE]  # (512, 4096)
    return np.ascontiguousarray(
        xs.reshape(ROWS_PER_CORE, NSUPER, 128).transpose(2, 1, 0)
    ).reshape(128, FREE)


def _unshard(yb):
    """Invert _shard for one core's output: (128, FREE) -> (512, 4096)."""
    return (
        np.ascontiguousarray(
            yb.reshape(128, NSUPER, ROWS_PER_CORE).transpose(2, 1, 0)
        )
        .reshape(ROWS_PER_CORE, H)
    )


def _run(x, W_samp, W_init, **run_kwargs):
    x2d = np.asarray(x, dtype=np.float32).reshape(NB, H).astype(NPBF16)
    W1 = np.asarray(W_samp, dtype=np.float32)[:, 0, :]  # (8, 16)
    W2 = np.asarray(W_init, dtype=np.float32)[:, :, 0]  # (16, 8)
    M = W2 @ W1  # (16, 16)
    K = np.ascontiguousarray(
        np.kron(np.eye(SP, dtype=np.float32), M.T)
    ).astype(NPBF16)

    nc = _get_nc()
    in_maps = [{"x": _shard(x2d, i), "k": K} for i in range(N_CORES)]
    res = run_bass_kernel_spmd(nc, in_maps, list(range(N_CORES)), **run_kwargs)
    out = np.concatenate(
        [_unshard(np.asarray(res.results[i]["y"])) for i in range(N_CORES)], axis=0
    ).astype(np.float32)
    return out.reshape(NB, H, 1), res


def kernel(x, W_samp, W_init):
    out, _ = _run(x, W_samp, W_init)
    return out


# revision 3
# speedup vs baseline: 1.0217x; 1.0217x over previous
"""Trainium2 Bass kernel for nn_LSM_IniReconNet.

The reference computes, per contiguous 16-element block of the signal,
z = W1 @ block then y = W2 @ z — i.e. a fixed 16x16 linear map
M = W2 @ W1 applied blockwise. This is pure streaming (memory-bound):
every element is read once, transformed by M, written once.

v2 strategy:
  * bf16 on the wire both directions (rel-err gate is 2e-2; bf16
    end-to-end lands ~4e-3), halving HBM traffic per core to
    4 MB in + 4 MB out.
  * The host lays each core's slice out as [128 partitions = signal
    position within a 128-superblock, free = (superblock, row)] so the
    contraction dim is already on partitions: the device needs NO
    transposes — just DMA in, one bf16 matmul per [128,512] chunk
    against the constant K = kron(I8, M.T), a PSUM->SBUF copy (casting
    back to bf16), and DMA out. The host inverts the permutation.
  * HWDGE DMAs: loads on nc.sync (SP ring), stores on nc.scalar (ACT
    ring) — separate rings, concurrent in/out streaming, ~0.6us fixed
    cost vs ~2us for the SWDGE path.

Sharding: pure data parallel — batch rows split across 8 cores, K
replicated.
"""

import sys

for _p in ("/opt/trn_rl_repo", "/root/.axon_site/_ro/trn_rl_repo"):
    if _p not in sys.path:
        sys.path.insert(0, _p)

import ml_dtypes
import numpy as np

import concourse.bass as bass
import concourse.mybir as mybir
from concourse.bass_utils import run_bass_kernel_spmd
from concourse.tile import TileContext

F32 = mybir.dt.float32
BF16 = mybir.dt.bfloat16
NPBF16 = np.dtype(ml_dtypes.bfloat16)

NB = 4096  # batch
H = 4096  # signal length
BLOCK = 16
SP = 8
N_CORES = 8
ROWS_PER_CORE = NB // N_CORES  # 512
NSUPER = H // 128  # 32 superblocks of 128 positions per row
NGROUPS = 4  # DMA granularity: 1 MB bf16 per group
CHUNKS_PER_GROUP = (NSUPER * ROWS_PER_CORE // 512) // NGROUPS  # 8
FREE = NSUPER * ROWS_PER_CORE  # 16384 free columns on chip

_NC_CACHE = {}


def _split_multi_waits(nc):
    """walrus codegen accepts at most one semaphore wait per instruction
    (beyond what same-queue elision removes). Tile attaches several — most
    notably on the kernel-tail drain. Hoist all but one wait onto wait-only
    NOPs placed immediately before the instruction on the same engine queue.
    """
    ctr = 0
    for fn in nc.m.functions:
        for blk in fn.blocks:
            old = list(blk.instructions)
            if not any(
                i.sync_info is not None and len(i.sync_info.on_wait) > 1 for i in old
            ):
                continue
            new = []
            for inst in old:
                si = inst.sync_info
                if si is not None and len(si.on_wait) > 1:
                    waits = list(si.on_wait)
                    for w in waits[:-1]:
                        ctr += 1
                        new.append(
                            mybir.InstNoOp(
                                name=f"I-waitsplit-{ctr}",
                                sync_info=mybir.SyncInfo(on_wait=[w], on_update=[]),
                                bass_nofuse=True,
                                engine=inst.engine,
                            )
                        )
                    inst.sync_info = mybir.SyncInfo(
                        on_wait=[waits[-1]], on_update=list(si.on_update)
                    )
                new.append(inst)
            blk.instructions = new
    return nc


def _build():
    """Per-core SPMD program.

    x: (128, FREE) bf16 — partition p holds position (128*c + p) of the
    signal for superblock c, free col c*512+n is batch row n.
    k: (128, 128) bf16 = kron(I8, M.T).  y: same layout as x.
    """
    nc = bass.Bass()
    x = nc.declare_dram_parameter("x", [128, FREE], BF16, isOutput=False)
    k = nc.declare_dram_parameter("k", [128, 128], BF16, isOutput=False)
    y = nc.declare_dram_parameter("y", [128, FREE], BF16, isOutput=True)
    gcols = FREE // NGROUPS  # 4096 free cols per DMA group

    with TileContext(nc) as tc:
        with (
            tc.tile_pool(name="kpool", bufs=1) as kp,
            tc.tile_pool(name="xin", bufs=3) as xin,
            tc.tile_pool(name="yout", bufs=4) as yp,
            tc.tile_pool(name="ps", bufs=2, space="PSUM") as pp,
        ):
            k_sb = kp.tile([128, 128], BF16)
            nc.sync.dma_start(out=k_sb[:], in_=k[:])
            # Warm-up matmul: consumes the K-DMA wait early.
            ps = pp.tile([128, 2048], F32, tag="ps")
            nc.tensor.matmul(ps[:, :128], k_sb[:], k_sb[:], start=True, stop=True)
            for g in range(NGROUPS):
                xt = xin.tile([128, gcols], BF16)
                nc.sync.dma_start(out=xt[:], in_=x[:, g * gcols : (g + 1) * gcols])
                yt = yp.tile([128, gcols], BF16)
                # 4-bank PSUM tiles: 4 matmuls each, then ONE wide DVE copy
                # (PSUM fp32 -> SBUF bf16) to amortize per-instr overhead.
                for h in range(gcols // 2048):
                    ps = pp.tile([128, 2048], F32, tag="ps")
                    for c in range(4):
                        nc.tensor.matmul(
                            ps[:, c * 512 : (c + 1) * 512],
                            k_sb[:],
                            xt[:, h * 2048 + c * 512 : h * 2048 + (c + 1) * 512],
                            start=True,
                            stop=True,
                        )
                    nc.vector.tensor_copy(
                        yt[:, h * 2048 : (h + 1) * 2048], ps[:]
                    )
                nc.scalar.dma_start(out=y[:, g * gcols : (g + 1) * gcols], in_=yt[:])
    return _split_multi_waits(nc)


def _get_nc():
    if "nc" not in _NC_CACHE:
        _NC_CACHE["nc"] = _build()
    return _NC_CACHE["nc"]


def _shard(x2d_bf16, i):
    """Core i's slice, permuted to device layout B[p, c*512+n] =
    x[n, 128c+p]."""
    xs = x2d_bf16[i * ROWS_PER_CORE : (i + 1) * ROWS_PER_CORE]  # (512, 4096)
    return np.ascontiguousarray(
        xs.reshape(ROWS_PER_CORE, NSUPER, 128).transpose(2, 1, 0)
    ).reshape(128, FREE)


def _unshard(yb):
    """Invert _shard for one core's output: (128, FREE) -> (512, 4096)."""
    return (
        np.ascontiguousarray(
            yb.reshape(128, NSUPER, ROWS_PER_CORE).transpose(2, 1, 0)
        )
        .reshape(ROWS_PER_CORE, H)
    )


def _run(x, W_samp, W_init, **run_kwargs):
    x2d = np.asarray(x, dtype=np.float32).reshape(NB, H).astype(NPBF16)
    W1 = np.asarray(W_samp, dtype=np.float32)[:, 0, :]  # (8, 16)
    W2 = np.asarray(W_init, dtype=np.float32)[:, :, 0]  # (16, 8)
    M = W2 @ W1  # (16, 16)
    K = np.ascontiguousarray(
        np.kron(np.eye(SP, dtype=np.float32), M.T)
    ).astype(NPBF16)

    nc = _get_nc()
    in_maps = [{"x": _shard(x2d, i), "k": K} for i in range(N_CORES)]
    res = run_bass_kernel_spmd(nc, in_maps, list(range(N_CORES)), **run_kwargs)
    out = np.concatenate(
        [_unshard(np.asarray(res.results[i]["y"])) for i in range(N_CORES)], axis=0
    ).astype(np.float32)
    return out.reshape(NB, H, 1), res


def kernel(x, W_samp, W_init):
    out, _ = _run(x, W_samp, W_init)
    return out


# revision 5
# speedup vs baseline: 1.1309x; 1.1069x over previous
"""Trainium2 Bass kernel for nn_LSM_IniReconNet.

The reference computes, per contiguous 16-element block of the signal,
z = W1 @ block then y = W2 @ z — i.e. a fixed 16x16 linear map
M = W2 @ W1 applied blockwise. This is pure streaming (memory-bound):
every element is read once, transformed by M, written once.

v2 strategy:
  * bf16 on the wire both directions (rel-err gate is 2e-2; bf16
    end-to-end lands ~4e-3), halving HBM traffic per core to
    4 MB in + 4 MB out.
  * The host lays each core's slice out as [128 partitions = signal
    position within a 128-superblock, free = (superblock, row)] so the
    contraction dim is already on partitions: the device needs NO
    transposes — just DMA in, one bf16 matmul per [128,512] chunk
    against the constant K = kron(I8, M.T), a PSUM->SBUF copy (casting
    back to bf16), and DMA out. The host inverts the permutation.
  * HWDGE DMAs: loads on nc.sync (SP ring), stores on nc.scalar (ACT
    ring) — separate rings, concurrent in/out streaming, ~0.6us fixed
    cost vs ~2us for the SWDGE path.

Sharding: pure data parallel — batch rows split across 8 cores, K
replicated.
"""

import sys

for _p in ("/opt/trn_rl_repo", "/root/.axon_site/_ro/trn_rl_repo"):
    if _p not in sys.path:
        sys.path.insert(0, _p)

import ml_dtypes
import numpy as np

import concourse.bass as bass
import concourse.mybir as mybir
from concourse.bass_utils import run_bass_kernel_spmd
from concourse.tile import TileContext

F32 = mybir.dt.float32
BF16 = mybir.dt.bfloat16
NPBF16 = np.dtype(ml_dtypes.bfloat16)

NB = 4096  # batch
H = 4096  # signal length
BLOCK = 16
SP = 8
N_CORES = 8
ROWS_PER_CORE = NB // N_CORES  # 512
NSUPER = H // 128  # 32 superblocks of 128 positions per row
NGROUPS = 4  # DMA granularity: 1 MB bf16 per group
CHUNKS_PER_GROUP = (NSUPER * ROWS_PER_CORE // 512) // NGROUPS  # 8
FREE = NSUPER * ROWS_PER_CORE  # 16384 free columns on chip

_NC_CACHE = {}


def _split_multi_waits(nc):
    """walrus codegen accepts at most one semaphore wait per instruction
    (beyond what same-queue elision removes). Tile attaches several — most
    notably on the kernel-tail drain. Hoist all but one wait onto wait-only
    NOPs placed immediately before the instruction on the same engine queue.
    """
    ctr = 0
    for fn in nc.m.functions:
        for blk in fn.blocks:
            old = list(blk.instructions)
            if not any(
                i.sync_info is not None and len(i.sync_info.on_wait) > 1 for i in old
            ):
                continue
            new = []
            for inst in old:
                si = inst.sync_info
                if si is not None and len(si.on_wait) > 1:
                    waits = list(si.on_wait)
                    for w in waits[:-1]:
                        ctr += 1
                        new.append(
                            mybir.InstNoOp(
                                name=f"I-waitsplit-{ctr}",
                                sync_info=mybir.SyncInfo(on_wait=[w], on_update=[]),
                                bass_nofuse=True,
                                engine=inst.engine,
                            )
                        )
                    inst.sync_info = mybir.SyncInfo(
                        on_wait=[waits[-1]], on_update=list(si.on_update)
                    )
                new.append(inst)
            blk.instructions = new
    return nc


def _build():
    """Per-core SPMD program.

    x: (128, FREE) bf16 — partition p holds position (128*c + p) of the
    signal for superblock c, free col c*512+n is batch row n.
    k: (128, 128) bf16 = kron(I8, M.T).  y: same layout as x.
    """
    nc = bass.Bass()
    x = nc.declare_dram_parameter("x", [128, FREE], BF16, isOutput=False)
    k = nc.declare_dram_parameter("k", [128, 128], BF16, isOutput=False)
    y = nc.declare_dram_parameter("y", [128, FREE], BF16, isOutput=True)
    gcols = FREE // NGROUPS  # 4096 free cols per DMA group

    with TileContext(nc) as tc:
        with (
            tc.tile_pool(name="kpool", bufs=1) as kp,
            tc.tile_pool(name="xin", bufs=3) as xin,
            tc.tile_pool(name="yout", bufs=4) as yp,
            tc.tile_pool(name="ps", bufs=2, space="PSUM") as pp,
        ):
            k_sb = kp.tile([128, 128], BF16)
            nc.sync.dma_start(out=k_sb[:], in_=k[:])
            # Warm-up matmul: consumes the K-DMA wait early.
            ps = pp.tile([128, 2048], F32, tag="ps")
            nc.tensor.matmul(ps[:, :128], k_sb[:], k_sb[:], start=True, stop=True)
            hh = 0
            for g in range(NGROUPS):
                xt = xin.tile([128, gcols], BF16)
                nc.sync.dma_start(out=xt[:], in_=x[:, g * gcols : (g + 1) * gcols])
                yt = yp.tile([128, gcols], BF16)
                # 4-bank PSUM tiles: 4 matmuls each, then ONE wide copy
                # (PSUM fp32 -> SBUF bf16), alternating DVE / ScalarE so the
                # two PSUM-capable engines drain banks in parallel. Each
                # half-group's 512 KB goes out immediately on the gpsimd
                # (SWDGE) queue — separate from the sync (HWDGE) input ring —
                # so the output stream overlaps the input stream.
                for h in range(gcols // 2048):
                    ps = pp.tile([128, 2048], F32, tag="ps")
                    for c in range(4):
                        nc.tensor.matmul(
                            ps[:, c * 512 : (c + 1) * 512],
                            k_sb[:],
                            xt[:, h * 2048 + c * 512 : h * 2048 + (c + 1) * 512],
                            start=True,
                            stop=True,
                        )
                    if hh % 2 == 0:
                        nc.vector.tensor_copy(yt[:, h * 2048 : (h + 1) * 2048], ps[:])
                    else:
                        nc.scalar.copy(yt[:, h * 2048 : (h + 1) * 2048], ps[:])
                    nc.gpsimd.dma_start(
                        out=y[:, g * gcols + h * 2048 : g * gcols + (h + 1) * 2048],
                        in_=yt[:, h * 2048 : (h + 1) * 2048],
                    )
                    hh += 1
    return _split_multi_waits(nc)


def _get_nc():
    if "nc" not in _NC_CACHE:
        _NC_CACHE["nc"] = _build()
    return _NC_CACHE["nc"]


def _shard(x2d_bf16, i):
    """Core i's slice, permuted to device layout B[p, c*512+n] =
    x[n, 128c+p]."""
    xs = x2d_bf16[i * ROWS_PER_CORE : (i + 1) * ROWS_PER_CORE]  # (512, 4096)
    return np.ascontiguousarray(
        xs.reshape(ROWS_PER_CORE, NSUPER, 128).transpose(2, 1, 0)
    ).reshape(128, FREE)


def _unshard(yb):
    """Invert _shard for one core's output: (128, FREE) -> (512, 4096)."""
    return (
        np.ascontiguousarray(
            yb.reshape(128, NSUPER, ROWS_PER_CORE).transpose(2, 1, 0)
        )
        .reshape(ROWS_PER_CORE, H)
    )


def _run(x, W_samp, W_init, **run_kwargs):
    x2d = np.asarray(x, dtype=np.float32).reshape(NB, H).astype(NPBF16)
    W1 = np.asarray(W_samp, dtype=np.float32)[:, 0, :]  # (8, 16)
    W2 = np.asarray(W_init, dtype=np.float32)[:, :, 0]  # (16, 8)
    M = W2 @ W1  # (16, 16)
    K = np.ascontiguousarray(
        np.kron(np.eye(SP, dtype=np.float32), M.T)
    ).astype(NPBF16)

    nc = _get_nc()
    in_maps = [{"x": _shard(x2d, i), "k": K} for i in range(N_CORES)]
    res = run_bass_kernel_spmd(nc, in_maps, list(range(N_CORES)), **run_kwargs)
    out = np.concatenate(
        [_unshard(np.asarray(res.results[i]["y"])) for i in range(N_CORES)], axis=0
    ).astype(np.float32)
    return out.reshape(NB, H, 1), res


def kernel(x, W_samp, W_init):
    out, _ = _run(x, W_samp, W_init)
    return out
